# revision 22
# baseline (speedup 1.0000x reference)
"""AttentionSuper (AutoFormer 2D rel-pos attention) Trainium2 Bass kernel.

B=64,N=197,C=640,H=10,D=64 ; data-parallel over batch: 8 batches/core x 8 cores.
v2: all rel-pos gathers/scatters are PE matmuls (no strided gather DMAs).
Layouts per batch:
  qkT[m] (128, 1576) feature-on-partition; v per batch (tok, 640)
  svb_v (30, 10*197) head-major; svb_hcm (30, 10*196) column-major grid order
  Gv_blk/Ghcm_blk (15, 14*140) block-major bias tables built by Sel matmuls
  scores^T per (b,h) with bias added via EF2v/EF2h matmuls (strided rhs APs)
  block sums bstb_v/bstb_hcm -> rel-pos-v applied via shifted-table matmuls
  cls-key rpv contribution fused into att@v psum; cls-query fixed on host
"""

import os
import numpy as np
import ml_dtypes

B, N, C = 64, 197, 640
H, D = 10, 64
NB = 8
NCORES = 8
NT = NB * N            # 1576 tokens per core
S = 14                 # spatial grid side
SCALE = D ** -0.5
CH4 = [(0, 394), (394, 394), (788, 394), (1182, 394)]
WIN = [(0, 3), (3, 3), (6, 3), (9, 3), (12, 2)]  # q-block windows


def _consts():
    kr = np.arange(196) // 14
    kc = np.arange(196) % 14
    # merged bias expansion: rows 0:15 v-blocks+cls, rows 32:47 h-blocks+cls
    EF2vh = np.zeros((79, 197), np.float32)
    EF2vh[kr, 1 + np.arange(196)] = 1
    EF2vh[14, 0] = 1
    EF2vh[64 + kc, 1 + np.arange(196)] = 1
    EF2vh[78, 0] = 1
    # merged block sums: cols 0:14 v-blocks, col 32 rowsum, cols 64:78 h-blocks
    E2all = np.zeros((197, 78), np.float32)
    E2all[1:, :][np.arange(196), kr] = 1
    E2all[:, 32] = 1
    E2all[1:, :][np.arange(196), 64 + kc] = 1
    # merged G selection: (62, 47*14); maps svb2 rows (0:30 sv, 32:62 sh)
    Sel2 = np.zeros((62, 79 * 14), np.float32)
    for r in range(14):
        for g in range(14):
            Sel2[15 - r + g, r * 79 + g] = 1
            Sel2[32 + 15 - r + g, r * 79 + 64 + g] = 1
        Sel2[0, r * 79 + 14] = 1
        Sel2[32, r * 79 + 78] = 1
    return EF2vh, E2all, Sel2


def _build_nc():
    import concourse.bass as bass  # noqa: F401
    import concourse.mybir as mybir
    from concourse import bacc
    from concourse.tile import TileContext
    from concourse.ap import AP

    f32 = mybir.dt.float32
    bf16 = mybir.dt.bfloat16
    EXP = mybir.ActivationFunctionType.Exp
    ADD = mybir.AluOpType.add
    MUL = mybir.AluOpType.mult

    nc = bacc.Bacc("TRN2", target_bir_lowering=False, debug=False,
                   num_devices=NCORES)

    xT_d = nc.dram_tensor("xT", [C, NT], bf16, kind="ExternalInput")
    wqkvT_d = nc.dram_tensor("wqkvT", [C, 3 * C], bf16, kind="ExternalInput")
    wprojT_d = nc.dram_tensor("wprojT", [C, C], bf16, kind="ExternalInput")
    bproj_d = nc.dram_tensor("bproj", [C, 1], f32, kind="ExternalInput")
    rpkT2_d = nc.dram_tensor("rpkT2", [128, 128], bf16, kind="ExternalInput")
    rpkT2h_d = nc.dram_tensor("rpkT2h", [128, 128], bf16, kind="ExternalInput")
    rpvs_d = nc.dram_tensor("rpvs", [78, 28 * 64], bf16, kind="ExternalInput")
    rpvcls_d = nc.dram_tensor("rpvcls", [1, 640], bf16, kind="ExternalInput")
    EF2vh_d = nc.dram_tensor("EF2vh", [79, 197], bf16, kind="ExternalInput")
    E2all_d = nc.dram_tensor("E2all", [197, 78], bf16, kind="ExternalInput")
    Sel2_d = nc.dram_tensor("Sel2", [62, 79 * 14], bf16, kind="ExternalInput")
    yT_d = nc.dram_tensor("yT", [C, NT], bf16, kind="ExternalOutput")

    from contextlib import ExitStack
    with TileContext(nc) as tc, ExitStack() as es:
        es.enter_context(nc.allow_low_precision(reason="bf16 attention kernel"))
        cp = es.enter_context(tc.tile_pool(name="const", bufs=1))
        qp = es.enter_context(tc.tile_pool(name="qk", bufs=1))
        op = es.enter_context(tc.tile_pool(name="outacc", bufs=1))
        vp = es.enter_context(tc.tile_pool(name="v", bufs=2))
        gp = es.enter_context(tc.tile_pool(name="g", bufs=2))
        bp = es.enter_context(tc.tile_pool(name="bst", bufs=2))
        rp_ = es.enter_context(tc.tile_pool(name="rp", bufs=2))
        avp = es.enter_context(tc.tile_pool(name="av", bufs=12))
        atp = es.enter_context(tc.tile_pool(name="at", bufs=6))
        rcpp = es.enter_context(tc.tile_pool(name="rcp", bufs=1))
        rcbp = es.enter_context(tc.tile_pool(name="rcb", bufs=2))
        sp = es.enter_context(tc.tile_pool(name="tmp", bufs=2))
        pj = es.enter_context(tc.tile_pool(name="pj", bufs=2, space="PSUM"))
        pq = es.enter_context(tc.tile_pool(name="pst", bufs=2, space="PSUM"))
        pm = es.enter_context(tc.tile_pool(name="psm", bufs=4, space="PSUM"))

        # ---- load constants / weights ----
        wq = [cp.tile([128, 3 * C], bf16, tag=f"wq{c}") for c in range(5)]
        wp = [cp.tile([128, C], bf16, tag=f"wp{c}") for c in range(5)]
        xT = [cp.tile([128, NT], bf16, tag=f"xT{c}") for c in range(5)]
        bpr = [cp.tile([128, 1], f32, tag=f"bp{c}") for c in range(5)]
        for c in range(5):
            nc.sync.dma_start(out=xT[c][:], in_=xT_d[128 * c:128 * (c + 1), :])
            nc.scalar.dma_start(out=wq[c][:], in_=wqkvT_d[128 * c:128 * (c + 1), :])
            nc.sync.dma_start(out=wp[c][:], in_=wprojT_d[128 * c:128 * (c + 1), :])
            nc.scalar.dma_start(out=bpr[c][:], in_=bproj_d[128 * c:128 * (c + 1), :])
        rpkT2 = cp.tile([128, 128], bf16)
        rpkT2h = cp.tile([128, 128], bf16)
        rpvs = cp.tile([78, 28 * 64], bf16)
        rpvcls = cp.tile([1, 640], bf16)
        EF2vh = cp.tile([79, 197], bf16)
        E2all_a = cp.tile([128, 78], bf16)
        E2all_b = cp.tile([69, 78], bf16)
        Sel2 = cp.tile([62, 79 * 14], bf16)
        nc.sync.dma_start(out=rpkT2[:], in_=rpkT2_d[:])
        nc.sync.dma_start(out=rpkT2h[:], in_=rpkT2h_d[:])
        nc.sync.dma_start(out=rpvs[:], in_=rpvs_d[:])
        nc.sync.dma_start(out=rpvcls[:], in_=rpvcls_d[:])
        nc.sync.dma_start(out=EF2vh[:], in_=EF2vh_d[:])
        nc.sync.dma_start(out=E2all_a[:], in_=E2all_d[0:128, :])
        nc.sync.dma_start(out=E2all_b[:], in_=E2all_d[128:197, :])
        nc.sync.dma_start(out=Sel2[:], in_=Sel2_d[:])
        ones32 = cp.tile([33, 128], bf16)
        nc.gpsimd.memset(ones32[:], 1.0)

        # ---- qk projection: qkT[m][o, tok] ----
        qkT = [qp.tile([128, NT], bf16, tag=f"qkT{m}") for m in range(10)]
        cp_eng = [nc.scalar, nc.vector, nc.gpsimd]
        for m in range(10):
            for ci, (o0, on) in enumerate(CH4):
                acc = pj.tile([128, on], f32, tag="pj")
                for c in range(5):
                    nc.tensor.matmul(
                        acc[:], wq[c][:, 128 * m:128 * (m + 1)],
                        xT[c][:, o0:o0 + on],
                        start=(c == 0), stop=(c == 4))
                eng = cp_eng[(m * 4 + ci) % 3]
                if eng is nc.scalar:
                    eng.copy(out=qkT[m][:, o0:o0 + on], in_=acc[:])
                else:
                    eng.tensor_copy(out=qkT[m][:, o0:o0 + on], in_=acc[:])

        outT = [op.tile([128, NT], bf16, tag=f"oT{m}") for m in range(5)]

        for b in range(NB):
            t0 = b * N
            # ---- v projection: (tok, 640) ----
            vt = [vp.tile([128, C], bf16, tag="v0"), vp.tile([69, C], bf16, tag="v1")]
            toksl = [(t0, 128), (t0 + 128, 69)]
            for s in range(2):
                ts, tn = toksl[s]
                for (o0, on) in [(0, 512), (512, 128)]:
                    acc = pj.tile([tn, on], f32, tag="pj")
                    for c in range(5):
                        nc.tensor.matmul(
                            acc[:], xT[c][:, ts:ts + tn],
                            wq[c][:, 2 * C + o0:2 * C + o0 + on],
                            start=(c == 0), stop=(c == 4))
                    nc.scalar.copy(out=vt[s][:, o0:o0 + on], in_=acc[:])
            nc.gpsimd.tensor_tensor(out=vt[0][0:1, :], in0=vt[0][0:1, :],
                                    in1=rpvcls[:], op=ADD)

            # ---- sv pass: svb_v (30, 1970) rm head-major, svb_hcm (30, 1960) cm ----
            svb_v = svp.tile([30, 1970], bf16, tag="svv")
            svb_hcm = svp.tile([30, 1960], bf16, tag="svh")
            for i in range(5):
                s_ps = pm.tile([128, 197], f32, tag="sm")
                nc.tensor.matmul(s_ps[:], rpkT2[:], qkT[i][:, t0:t0 + 197],
                                 start=True, stop=True)
                nc.vector.tensor_copy(out=svb_v[:, (2 * i) * 197:(2 * i + 1) * 197],
                                      in_=s_ps[0:30, :])
                nc.vector.tensor_copy(out=svb_v[:, (2 * i + 1) * 197:(2 * i + 2) * 197],
                                      in_=s_ps[64:94, :])
                s_cm = pm.tile([128, 197], f32, tag="sm")
                rhs_cm = AP(qkT[i][:].tensor, t0 + 1,
                            [[NT, 128], [1, 14], [14, 14]])
                nc.tensor.matmul(s_cm[:, 0:196], rpkT2h[:], rhs_cm,
                                 start=True, stop=True)
                nc.scalar.copy(out=svb_hcm[:, (2 * i) * 196:(2 * i + 1) * 196],
                               in_=s_cm[0:30, 0:196])
                nc.scalar.copy(out=svb_hcm[:, (2 * i + 1) * 196:(2 * i + 2) * 196],
                               in_=s_cm[64:94, 0:196])

            # ---- G build: Gv_blk/Ghcm_blk (15, 1960), col = blk*140 + h*14 + j ----
            Gv_blk = gp.tile([15, 1960], bf16, tag="gv")
            Ghcm_blk = gp.tile([15, 1960], bf16, tag="gh")
            for (r0, nr) in WIN:
                gps = pj.tile([15, 512], f32, tag="pj")
                for j in range(nr):
                    r = r0 + j
                    rhs = AP(svb_v[:].tensor, 1 + 14 * r,
                             [[1970, 30], [197, 10], [1, 14]])
                    nc.tensor.matmul(gps[:, j * 140:(j + 1) * 140],
                                     Sel[:, r * 15:r * 15 + 15], rhs,
                                     start=True, stop=True)
                nc.vector.tensor_copy(
                    out=Gv_blk[:, r0 * 140:(r0 + nr) * 140],
                    in_=gps[:, 0:nr * 140])
                ghs = pj.tile([15, 512], f32, tag="pj")
                for j in range(nr):
                    r = r0 + j
                    rhs = AP(svb_hcm[:].tensor, 14 * r,
                             [[1960, 30], [196, 10], [1, 14]])
                    nc.tensor.matmul(ghs[:, j * 140:(j + 1) * 140],
                                     Sel[:, r * 15:r * 15 + 15], rhs,
                                     start=True, stop=True)
                nc.scalar.copy(
                    out=Ghcm_blk[:, r0 * 140:(r0 + nr) * 140],
                    in_=ghs[:, 0:nr * 140])

            # ---- pass 2 per head ----
            bstb_v = bp.tile([14, 1970], bf16, tag="bv")
            bstb_hcm = bp.tile([14, 1960], bf16, tag="bh")
            rcp_f = rcpp.tile([1, 1970], f32, tag="rcf")
            avs = [None] * H
            for h in range(H):
                m = h // 2
                kp = (h % 2) * 64
                qT = qkT[m][kp:kp + 64, t0:t0 + 197]
                kTt = qkT[5 + m]
                at = [atp.tile([128, 197], bf16, tag="at0"),
                      atp.tile([69, 197], bf16, tag="at1")]
                for s, (k0, kn) in enumerate([(0, 128), (128, 69)]):
                    st = pq.tile([kn, 197], f32, tag="st")
                    nc.tensor.matmul(st[:], kTt[kp:kp + 64, t0 + k0:t0 + k0 + kn],
                                     qT, start=True, stop=False)
                    rhv = AP(Gv_blk[:].tensor, 14 * h,
                             [[1960, 15], [140, 14], [1, 14]])
                    nc.tensor.matmul(st[:, 1:197], EF2v[:, k0:k0 + kn], rhv,
                                     start=False, stop=False,
                                     skip_group_check=True)
                    rhh = AP(Ghcm_blk[:].tensor, 14 * h,
                             [[1960, 15], [1, 14], [140, 14]])
                    nc.tensor.matmul(st[:, 1:197], EF2h[:, k0:k0 + kn], rhh,
                                     start=False, stop=True,
                                     skip_group_check=True)
                    nc.scalar.activation(out=at[s][:], in_=st[:], func=EXP,
                                         scale=SCALE)
                # block sums (v, rm) + rowsum
                bv = pm.tile([14, 197], f32, tag="sm")
                nc.tensor.matmul(bv[:], E2vb_a[:], at[0][:], start=True, stop=False)
                nc.tensor.matmul(bv[:], E2vb_b[:], at[1][:], start=False, stop=True)
                nc.vector.tensor_copy(out=bstb_v[:, h * 197:(h + 1) * 197], in_=bv[:])
                # block sums (h, cm)
                bh = pm.tile([14, 197], f32, tag="sm")
                cm0 = AP(at[0][:].tensor, 1, [[197, 128], [1, 14], [14, 14]])
                cm1 = AP(at[1][:].tensor, 1, [[197, 69], [1, 14], [14, 14]])
                nc.tensor.matmul(bh[:, 0:196], E2hb_a[:], cm0, start=True, stop=False)
                nc.tensor.matmul(bh[:, 0:196], E2hb_b[:], cm1, start=False, stop=True)
                nc.scalar.copy(out=bstb_hcm[:, h * 196:(h + 1) * 196],
                               in_=bh[:, 0:196])
                # rowsum -> reciprocal
                rs = pm.tile([1, 197], f32, tag="sm")
                nc.tensor.matmul(rs[:], ones_c[0:128, :], at[0][:], start=True, stop=False)
                nc.tensor.matmul(rs[:], ones_c[0:69, :], at[1][:], start=False, stop=True)
                nc.vector.reciprocal_approx_fast(
                    out=rcp_f[0:1, h * 197:(h + 1) * 197], in_=rs[:])
                # att @ v (+ cls-key rel-pos-v fused)
                av = pm.tile([64, 197], f32, tag="sm")
                nc.tensor.matmul(av[:], vt[0][:, h * 64:h * 64 + 64], at[0][:],
                                 start=True, stop=False)
                nc.tensor.matmul(av[:], vt[1][:, h * 64:h * 64 + 64], at[1][:],
                                 start=False, stop=False)
                nc.tensor.matmul(av[:], rpvcls[:], at[0][0:1, :],
                                 start=False, stop=True)
                avsb = avp.tile([64, 197], bf16, tag="avs")
                nc.scalar.copy(out=avsb[:], in_=av[:])
                avs[h] = avsb

            rcp_b = rcbp.tile([1, 1970], bf16, tag="rcb")
            nc.scalar.copy(out=rcp_b[:], in_=rcp_f[:])

            # ---- rel-pos-v apply: rp_v_hm / rp_h_hm (64, 1960) ----
            rp_v = rp_.tile([64, 1960], bf16, tag="rv")
            rp_h = rp_.tile([64, 1960], bf16, tag="rh")
            for (r0, nr) in WIN:
                rv = pj.tile([64, 512], f32, tag="pj")
                for j in range(nr):
                    r = r0 + j
                    rhs = AP(bstb_v[:].tensor, 1 + 14 * r,
                             [[1970, 14], [197, 10], [1, 14]])
                    nc.tensor.matmul(rv[:, j * 140:(j + 1) * 140],
                                     rpvs[:, r * 64:(r + 1) * 64], rhs,
                                     start=True, stop=True)
                nc.vector.tensor_copy(out=rp_v[:, r0 * 140:(r0 + nr) * 140],
                                      in_=rv[:, 0:nr * 140])
                rh = pj.tile([64, 512], f32, tag="pj")
                for j in range(nr):
                    r = r0 + j
                    rhs = AP(bstb_hcm[:].tensor, 14 * r,
                             [[1960, 14], [196, 10], [1, 14]])
                    nc.tensor.matmul(rh[:, j * 140:(j + 1) * 140],
                                     rpvs[:, 896 + r * 64:896 + (r + 1) * 64], rhs,
                                     start=True, stop=True)
                nc.scalar.copy(out=rp_h[:, r0 * 140:(r0 + nr) * 140],
                               in_=rh[:, 0:nr * 140])

            # ---- combine + normalize ----
            for h in range(H):
                m = h // 2
                kp = (h % 2) * 64
                rb = pq.tile([64, 197], f32, tag="st")
                nc.tensor.matmul(rb[:], ones_r[:],
                                 rcp_b[0:1, h * 197:(h + 1) * 197],
                                 start=True, stop=True)
                inv = AP(rp_v[:].tensor, 14 * h, [[1960, 64], [140, 14], [1, 14]])
                inh = AP(rp_h[:].tensor, 14 * h, [[1960, 64], [1, 14], [140, 14]])
                t1 = sp.tile([64, 196], f32, tag="t1")
                nc.gpsimd.tensor_tensor(out=t1[:], in0=inv, in1=inh, op=ADD)
                t2 = sp.tile([64, 196], f32, tag="t2")
                nc.vector.tensor_tensor(out=t2[:], in0=avs[h][:, 1:197], in1=t1[:],
                                        op=ADD)
                dst = outT[m][kp:kp + 64, t0 + 1:t0 + 197]
                nc.vector.tensor_tensor(out=dst, in0=t2[:], in1=rb[:, 1:197],
                                        op=MUL)
                nc.vector.tensor_tensor(out=outT[m][kp:kp + 64, t0:t0 + 1],
                                        in0=avs[h][:, 0:1], in1=rb[:, 0:1],
                                        op=MUL)

        # ---- final projection yT = wprojT.T @ outT + bproj ----
        out_eng = [nc.sync, nc.scalar]
        for m in range(5):
            for ci, (o0, on) in enumerate(CH4):
                acc = pj.tile([128, on], f32, tag="pj")
                for c in range(5):
                    nc.tensor.matmul(
                        acc[:], wp[c][:, 128 * m:128 * (m + 1)],
                        outT[c][:, o0:o0 + on],
                        start=(c == 0), stop=(c == 4))
                ysb = sp.tile([128, on], f32, tag="ysb")
                nc.vector.tensor_scalar_add(out=ysb[:], in0=acc[:],
                                            scalar1=bpr[m][:])
                out_eng[(m * 4 + ci) % 2].dma_start(
                    out=yT_d[128 * m:128 * (m + 1), o0:o0 + on], in_=ysb[:])

    nc.compile()
    return nc


_NC_CACHE = None


def kernel(x, w_qkv, w_proj, b_proj, rpk_v, rpk_h, rpv_v, rpv_h):
    global _NC_CACHE
    from concourse.bass_utils import run_bass_kernel_spmd

    if _NC_CACHE is None:
        _NC_CACHE = _build_nc()
    nc = _NC_CACHE

    EF2vh, E2all, Sel2 = _consts()
    wqkvT = np.ascontiguousarray(np.asarray(w_qkv).T).astype(ml_dtypes.bfloat16)
    wprojT = np.ascontiguousarray(np.asarray(w_proj).T).astype(ml_dtypes.bfloat16)
    bproj = np.asarray(b_proj, np.float32).reshape(C, 1)
    rpk_v = np.asarray(rpk_v, np.float32)
    rpk_h = np.asarray(rpk_h, np.float32)
    rpv_v = np.asarray(rpv_v, np.float32)
    rpv_h = np.asarray(rpv_h, np.float32)
    rpkT2s = np.concatenate([rpk_v.T, rpk_h.T], axis=1)
    rpkT2 = np.zeros((128, 128), ml_dtypes.bfloat16)
    rpkT2[0:64, 0:30] = rpk_v.T.astype(ml_dtypes.bfloat16)
    rpkT2[64:128, 64:94] = rpk_v.T.astype(ml_dtypes.bfloat16)
    rpkT2h = np.zeros((128, 128), ml_dtypes.bfloat16)
    rpkT2h[0:64, 32:62] = rpk_h.T.astype(ml_dtypes.bfloat16)
    rpkT2h[64:128, 96:126] = rpk_h.T.astype(ml_dtypes.bfloat16)
    rpvs = np.zeros((78, 28 * 64), np.float32)
    for r in range(14):
        rpvs[0:14, r * 64:(r + 1) * 64] = rpv_v[15 - r:29 - r, :]
        rpvs[64:78, 896 + r * 64:896 + (r + 1) * 64] = rpv_h[15 - r:29 - r, :]
    rpvcls = np.tile((rpv_v[0] + rpv_h[0]).reshape(1, 64), (1, 10))

    bf = ml_dtypes.bfloat16
    shared = {
        "wqkvT": wqkvT, "wprojT": wprojT, "bproj": bproj,
        "rpkT2": rpkT2, "rpkT2h": rpkT2h, "rpvs": rpvs.astype(bf), "rpvcls": rpvcls.astype(bf),
        "EF2vh": EF2vh.astype(bf), "E2all": E2all.astype(bf),
        "Sel2": Sel2.astype(bf),
    }
    x = np.asarray(x, np.float32)
    in_maps = []
    for i in range(NCORES):
        xs = x[i * NB:(i + 1) * NB].reshape(NT, C)
        xT = np.ascontiguousarray(xs.T).astype(bf)
        in_maps.append(dict(shared, xT=xT))

    trace = bool(os.environ.get("BASS_KERNEL_TRACE"))
    kw = {}
    if trace:
        kw = dict(trace=True, tmpdir=os.environ.get("BASS_KERNEL_TRACE_DIR") or None)
    res = run_bass_kernel_spmd(nc, in_maps, core_ids=list(range(NCORES)), **kw)
    kernel.last_result = res

    y = np.empty((B, N, C), np.float32)
    for i in range(NCORES):
        y[i * NB:(i + 1) * NB] = res.results[i]["yT"][:, :NT].T.reshape(NB, N, C)
    # cls-query rel-pos-v correction (constant across batch)
    rep = np.tile((rpv_v[0] + rpv_h[0]).astype(np.float32), H)
    y[:, 0, :] += np.asarray(w_proj, np.float32) @ rep
    return y


# revision 24
# speedup vs baseline: 1.0393x; 1.0393x over previous
"""AttentionSuper (AutoFormer 2D rel-pos attention) Trainium2 Bass kernel.

B=64,N=197,C=640,H=10,D=64 ; data-parallel over batch: 8 batches/core x 8 cores.
v2: all rel-pos gathers/scatters are PE matmuls (no strided gather DMAs).
Layouts per batch:
  qkT[m] (128, 1576) feature-on-partition; v per batch (tok, 640)
  svb_v (30, 10*197) head-major; svb_hcm (30, 10*196) column-major grid order
  Gv_blk/Ghcm_blk (15, 14*140) block-major bias tables built by Sel matmuls
  scores^T per (b,h) with bias added via EF2v/EF2h matmuls (strided rhs APs)
  block sums bstb_v/bstb_hcm -> rel-pos-v applied via shifted-table matmuls
  cls-key rpv contribution fused into att@v psum; cls-query fixed on host
"""

import os
import numpy as np
import ml_dtypes

B, N, C = 64, 197, 640
H, D = 10, 64
NB = 8
NCORES = 8
NT = NB * N            # 1576 tokens per core
S = 14                 # spatial grid side
SCALE = D ** -0.5
CH4 = [(0, 394), (394, 394), (788, 394), (1182, 394)]
WIN = [(0, 3), (3, 3), (6, 3), (9, 3), (12, 2)]  # q-block windows


def _consts():
    kr = np.arange(196) // 14
    kc = np.arange(196) % 14
    # merged bias expansion: rows 0:15 v-blocks+cls, rows 32:47 h-blocks+cls
    EF2vh = np.zeros((79, 197), np.float32)
    EF2vh[kr, 1 + np.arange(196)] = 1
    EF2vh[14, 0] = 1
    EF2vh[64 + kc, 1 + np.arange(196)] = 1
    EF2vh[78, 0] = 1
    # merged block sums: cols 0:14 v-blocks, col 32 rowsum, cols 64:78 h-blocks
    E2all = np.zeros((197, 78), np.float32)
    E2all[1:, :][np.arange(196), kr] = 1
    E2all[:, 32] = 1
    E2all[1:, :][np.arange(196), 64 + kc] = 1
    # merged G selection: (62, 47*14); maps svb2 rows (0:30 sv, 32:62 sh)
    Sel2 = np.zeros((62, 79 * 14), np.float32)
    for r in range(14):
        for g in range(14):
            Sel2[15 - r + g, r * 79 + g] = 1
            Sel2[32 + 15 - r + g, r * 79 + 64 + g] = 1
        Sel2[0, r * 79 + 14] = 1
        Sel2[32, r * 79 + 78] = 1
    return EF2vh, E2all, Sel2


def _build_nc():
    import concourse.bass as bass  # noqa: F401
    import concourse.mybir as mybir
    from concourse import bacc
    from concourse.tile import TileContext
    from concourse.ap import AP

    f32 = mybir.dt.float32
    bf16 = mybir.dt.bfloat16
    EXP = mybir.ActivationFunctionType.Exp
    ADD = mybir.AluOpType.add
    MUL = mybir.AluOpType.mult

    nc = bacc.Bacc("TRN2", target_bir_lowering=False, debug=False,
                   num_devices=NCORES)

    xT_d = nc.dram_tensor("xT", [C, NT], bf16, kind="ExternalInput")
    wqkvT_d = nc.dram_tensor("wqkvT", [C, 3 * C], bf16, kind="ExternalInput")
    wprojT_d = nc.dram_tensor("wprojT", [C, C], bf16, kind="ExternalInput")
    bproj_d = nc.dram_tensor("bproj", [C, 1], f32, kind="ExternalInput")
    rpkT2_d = nc.dram_tensor("rpkT2", [128, 128], bf16, kind="ExternalInput")
    rpkT2h_d = nc.dram_tensor("rpkT2h", [128, 128], bf16, kind="ExternalInput")
    rpvs_d = nc.dram_tensor("rpvs", [78, 28 * 64], bf16, kind="ExternalInput")
    rpvcls_d = nc.dram_tensor("rpvcls", [1, 640], bf16, kind="ExternalInput")
    EF2vh_d = nc.dram_tensor("EF2vh", [79, 197], bf16, kind="ExternalInput")
    E2all_d = nc.dram_tensor("E2all", [197, 78], bf16, kind="ExternalInput")
    Sel2_d = nc.dram_tensor("Sel2", [62, 79 * 14], bf16, kind="ExternalInput")
    yT_d = nc.dram_tensor("yT", [C, NT], bf16, kind="ExternalOutput")

    from contextlib import ExitStack
    with TileContext(nc) as tc, ExitStack() as es:
        es.enter_context(nc.allow_low_precision(reason="bf16 attention kernel"))
        cp = es.enter_context(tc.tile_pool(name="const", bufs=1))
        qp = es.enter_context(tc.tile_pool(name="qk", bufs=1))
        op = es.enter_context(tc.tile_pool(name="outacc", bufs=1))
        vp = es.enter_context(tc.tile_pool(name="v", bufs=2))
        gp = es.enter_context(tc.tile_pool(name="g", bufs=2))
        bp = es.enter_context(tc.tile_pool(name="bst", bufs=2))
        rp_ = es.enter_context(tc.tile_pool(name="rp", bufs=2))
        avp = es.enter_context(tc.tile_pool(name="av", bufs=12))
        atp = es.enter_context(tc.tile_pool(name="at", bufs=5))
        rcpp = es.enter_context(tc.tile_pool(name="rcp", bufs=1))
        rcbp = es.enter_context(tc.tile_pool(name="rcb", bufs=2))
        sp = es.enter_context(tc.tile_pool(name="tmp", bufs=2))
        pj = es.enter_context(tc.tile_pool(name="pj", bufs=3, space="PSUM"))
        pq = es.enter_context(tc.tile_pool(name="pst", bufs=2, space="PSUM"))
        pm = es.enter_context(tc.tile_pool(name="psm", bufs=3, space="PSUM"))

        # ---- load constants / weights ----
        wq = [cp.tile([128, 3 * C], bf16, tag=f"wq{c}") for c in range(5)]
        wp = [cp.tile([128, C], bf16, tag=f"wp{c}") for c in range(5)]
        xT = [cp.tile([128, NT], bf16, tag=f"xT{c}") for c in range(5)]
        bpr = [cp.tile([128, 1], f32, tag=f"bp{c}") for c in range(5)]
        for c in range(5):
            nc.sync.dma_start(out=xT[c][:], in_=xT_d[128 * c:128 * (c + 1), :])
            nc.scalar.dma_start(out=wq[c][:], in_=wqkvT_d[128 * c:128 * (c + 1), :])
            nc.sync.dma_start(out=wp[c][:], in_=wprojT_d[128 * c:128 * (c + 1), :])
            nc.scalar.dma_start(out=bpr[c][:], in_=bproj_d[128 * c:128 * (c + 1), :])
        rpkT2 = cp.tile([128, 128], bf16)
        rpkT2h = cp.tile([128, 128], bf16)
        rpvs = cp.tile([78, 28 * 64], bf16)
        rpvcls = cp.tile([1, 640], bf16)
        EF2vh = cp.tile([79, 197], bf16)
        E2all_a = cp.tile([128, 78], bf16)
        E2all_b = cp.tile([69, 78], bf16)
        Sel2 = cp.tile([62, 79 * 14], bf16)
        nc.sync.dma_start(out=rpkT2[:], in_=rpkT2_d[:])
        nc.sync.dma_start(out=rpkT2h[:], in_=rpkT2h_d[:])
        nc.sync.dma_start(out=rpvs[:], in_=rpvs_d[:])
        nc.sync.dma_start(out=rpvcls[:], in_=rpvcls_d[:])
        nc.sync.dma_start(out=EF2vh[:], in_=EF2vh_d[:])
        nc.sync.dma_start(out=E2all_a[:], in_=E2all_d[0:128, :])
        nc.sync.dma_start(out=E2all_b[:], in_=E2all_d[128:197, :])
        nc.sync.dma_start(out=Sel2[:], in_=Sel2_d[:])
        ones32 = cp.tile([33, 128], bf16)
        nc.gpsimd.memset(ones32[:], 1.0)

        # ---- qk projection: qkT[m][o, tok] ----
        qkT = [qp.tile([128, NT], bf16, tag=f"qkT{m}") for m in range(10)]
        cp_eng = [nc.scalar, nc.vector, nc.gpsimd]
        for m in range(10):
            for ci, (o0, on) in enumerate(CH4):
                acc = pj.tile([128, on], f32, tag="pj")
                for c in range(5):
                    nc.tensor.matmul(
                        acc[:], wq[c][:, 128 * m:128 * (m + 1)],
                        xT[c][:, o0:o0 + on],
                        start=(c == 0), stop=(c == 4))
                eng = cp_eng[(m * 4 + ci) % 3]
                if eng is nc.scalar:
                    eng.copy(out=qkT[m][:, o0:o0 + on], in_=acc[:])
                else:
                    eng.tensor_copy(out=qkT[m][:, o0:o0 + on], in_=acc[:])

        outT = [op.tile([128, NT], bf16, tag=f"oT{m}") for m in range(5)]

        for b in range(NB):
            t0 = b * N
            # ---- v projection: (tok, 640) ----
            vt = [vp.tile([128, C], bf16, tag="v0"), vp.tile([69, C], bf16, tag="v1")]
            toksl = [(t0, 128), (t0 + 128, 69)]
            for s in range(2):
                ts, tn = toksl[s]
                for (o0, on) in [(0, 512), (512, 128)]:
                    acc = pj.tile([tn, on], f32, tag="pj")
                    for c in range(5):
                        nc.tensor.matmul(
                            acc[:], xT[c][:, ts:ts + tn],
                            wq[c][:, 2 * C + o0:2 * C + o0 + on],
                            start=(c == 0), stop=(c == 4))
                    nc.scalar.copy(out=vt[s][:, o0:o0 + on], in_=acc[:])
            nc.gpsimd.tensor_tensor(out=vt[0][0:1, :], in0=vt[0][0:1, :],
                                    in1=rpvcls[:], op=ADD)

            # ---- sv pass: svb_v (30, 1970) rm head-major, svb_hcm (30, 1960) cm ----
            svb_v = svp.tile([30, 1970], bf16, tag="svv")
            svb_hcm = svp.tile([30, 1960], bf16, tag="svh")
            for i in range(5):
                s_ps = pm.tile([128, 197], f32, tag="sm")
                nc.tensor.matmul(s_ps[:], rpkT2[:], qkT[i][:, t0:t0 + 197],
                                 start=True, stop=True)
                nc.vector.tensor_copy(out=svb_v[:, (2 * i) * 197:(2 * i + 1) * 197],
                                      in_=s_ps[0:30, :])
                nc.vector.tensor_copy(out=svb_v[:, (2 * i + 1) * 197:(2 * i + 2) * 197],
                                      in_=s_ps[64:94, :])
                s_cm = pm.tile([128, 197], f32, tag="sm")
                rhs_cm = AP(qkT[i][:].tensor, t0 + 1,
                            [[NT, 128], [1, 14], [14, 14]])
                nc.tensor.matmul(s_cm[:, 0:196], rpkT2h[:], rhs_cm,
                                 start=True, stop=True)
                nc.scalar.copy(out=svb_hcm[:, (2 * i) * 196:(2 * i + 1) * 196],
                               in_=s_cm[0:30, 0:196])
                nc.scalar.copy(out=svb_hcm[:, (2 * i + 1) * 196:(2 * i + 2) * 196],
                               in_=s_cm[64:94, 0:196])

            # ---- G build: Gv_blk/Ghcm_blk (15, 1960), col = blk*140 + h*14 + j ----
            Gv_blk = gp.tile([15, 1960], bf16, tag="gv")
            Ghcm_blk = gp.tile([15, 1960], bf16, tag="gh")
            for (r0, nr) in WIN:
                gps = pj.tile([15, 512], f32, tag="pj")
                for j in range(nr):
                    r = r0 + j
                    rhs = AP(svb_v[:].tensor, 1 + 14 * r,
                             [[1970, 30], [197, 10], [1, 14]])
                    nc.tensor.matmul(gps[:, j * 140:(j + 1) * 140],
                                     Sel[:, r * 15:r * 15 + 15], rhs,
                                     start=True, stop=True)
                nc.vector.tensor_copy(
                    out=Gv_blk[:, r0 * 140:(r0 + nr) * 140],
                    in_=gps[:, 0:nr * 140])
                ghs = pj.tile([15, 512], f32, tag="pj")
                for j in range(nr):
                    r = r0 + j
                    rhs = AP(svb_hcm[:].tensor, 14 * r,
                             [[1960, 30], [196, 10], [1, 14]])
                    nc.tensor.matmul(ghs[:, j * 140:(j + 1) * 140],
                                     Sel[:, r * 15:r * 15 + 15], rhs,
                                     start=True, stop=True)
                nc.scalar.copy(
                    out=Ghcm_blk[:, r0 * 140:(r0 + nr) * 140],
                    in_=ghs[:, 0:nr * 140])

            # ---- pass 2 per head ----
            bstb_v = bp.tile([14, 1970], bf16, tag="bv")
            bstb_hcm = bp.tile([14, 1960], bf16, tag="bh")
            rcp_f = rcpp.tile([1, 1970], f32, tag="rcf")
            avs = [None] * H
            for h in range(H):
                m = h // 2
                kp = (h % 2) * 64
                qT = qkT[m][kp:kp + 64, t0:t0 + 197]
                kTt = qkT[5 + m]
                at = [atp.tile([128, 197], bf16, tag="at0"),
                      atp.tile([69, 197], bf16, tag="at1")]
                for s, (k0, kn) in enumerate([(0, 128), (128, 69)]):
                    st = pq.tile([kn, 197], f32, tag="st")
                    nc.tensor.matmul(st[:], kTt[kp:kp + 64, t0 + k0:t0 + k0 + kn],
                                     qT, start=True, stop=False)
                    rhv = AP(Gv_blk[:].tensor, 14 * h,
                             [[1960, 15], [140, 14], [1, 14]])
                    nc.tensor.matmul(st[:, 1:197], EF2v[:, k0:k0 + kn], rhv,
                                     start=False, stop=False,
                                     skip_group_check=True)
                    rhh = AP(Ghcm_blk[:].tensor, 14 * h,
                             [[1960, 15], [1, 14], [140, 14]])
                    nc.tensor.matmul(st[:, 1:197], EF2h[:, k0:k0 + kn], rhh,
                                     start=False, stop=True,
                                     skip_group_check=True)
                    nc.scalar.activation(out=at[s][:], in_=st[:], func=EXP,
                                         scale=SCALE)
                # block sums (v, rm) + rowsum
                bv = pm.tile([14, 197], f32, tag="sm")
                nc.tensor.matmul(bv[:], E2vb_a[:], at[0][:], start=True, stop=False)
                nc.tensor.matmul(bv[:], E2vb_b[:], at[1][:], start=False, stop=True)
                nc.vector.tensor_copy(out=bstb_v[:, h * 197:(h + 1) * 197], in_=bv[:])
                # block sums (h, cm)
                bh = pm.tile([14, 197], f32, tag="sm")
                cm0 = AP(at[0][:].tensor, 1, [[197, 128], [1, 14], [14, 14]])
                cm1 = AP(at[1][:].tensor, 1, [[197, 69], [1, 14], [14, 14]])
                nc.tensor.matmul(bh[:, 0:196], E2hb_a[:], cm0, start=True, stop=False)
                nc.tensor.matmul(bh[:, 0:196], E2hb_b[:], cm1, start=False, stop=True)
                nc.scalar.copy(out=bstb_hcm[:, h * 196:(h + 1) * 196],
                               in_=bh[:, 0:196])
                # rowsum -> reciprocal
                rs = pm.tile([1, 197], f32, tag="sm")
                nc.tensor.matmul(rs[:], ones_c[0:128, :], at[0][:], start=True, stop=False)
                nc.tensor.matmul(rs[:], ones_c[0:69, :], at[1][:], start=False, stop=True)
                nc.vector.reciprocal_approx_fast(
                    out=rcp_f[0:1, h * 197:(h + 1) * 197], in_=rs[:])
                # att @ v (+ cls-key rel-pos-v fused)
                av = pm.tile([64, 197], f32, tag="sm")
                nc.tensor.matmul(av[:], vt[0][:, h * 64:h * 64 + 64], at[0][:],
                                 start=True, stop=False)
                nc.tensor.matmul(av[:], vt[1][:, h * 64:h * 64 + 64], at[1][:],
                                 start=False, stop=False)
                nc.tensor.matmul(av[:], rpvcls[:], at[0][0:1, :],
                                 start=False, stop=True)
                avsb = avp.tile([64, 197], bf16, tag="avs")
                nc.scalar.copy(out=avsb[:], in_=av[:])
                avs[h] = avsb

            rcp_b = rcbp.tile([1, 1970], bf16, tag="rcb")
            nc.scalar.copy(out=rcp_b[:], in_=rcp_f[:])

            # ---- rel-pos-v apply: rp_v_hm / rp_h_hm (64, 1960) ----
            rp_v = rp_.tile([64, 1960], bf16, tag="rv")
            rp_h = rp_.tile([64, 1960], bf16, tag="rh")
            for (r0, nr) in WIN:
                rv = pj.tile([64, 512], f32, tag="pj")
                for j in range(nr):
                    r = r0 + j
                    rhs = AP(bstb_v[:].tensor, 1 + 14 * r,
                             [[1970, 14], [197, 10], [1, 14]])
                    nc.tensor.matmul(rv[:, j * 140:(j + 1) * 140],
                                     rpvs[:, r * 64:(r + 1) * 64], rhs,
                                     start=True, stop=True)
                nc.vector.tensor_copy(out=rp_v[:, r0 * 140:(r0 + nr) * 140],
                                      in_=rv[:, 0:nr * 140])
                rh = pj.tile([64, 512], f32, tag="pj")
                for j in range(nr):
                    r = r0 + j
                    rhs = AP(bstb_hcm[:].tensor, 14 * r,
                             [[1960, 14], [196, 10], [1, 14]])
                    nc.tensor.matmul(rh[:, j * 140:(j + 1) * 140],
                                     rpvs[:, 896 + r * 64:896 + (r + 1) * 64], rhs,
                                     start=True, stop=True)
                nc.scalar.copy(out=rp_h[:, r0 * 140:(r0 + nr) * 140],
                               in_=rh[:, 0:nr * 140])

            # ---- combine + normalize ----
            for h in range(H):
                m = h // 2
                kp = (h % 2) * 64
                rb = pq.tile([64, 197], f32, tag="st")
                nc.tensor.matmul(rb[:], ones_r[:],
                                 rcp_b[0:1, h * 197:(h + 1) * 197],
                                 start=True, stop=True)
                inv = AP(rp_v[:].tensor, 14 * h, [[1960, 64], [140, 14], [1, 14]])
                inh = AP(rp_h[:].tensor, 14 * h, [[1960, 64], [1, 14], [140, 14]])
                t1 = sp.tile([64, 196], f32, tag="t1")
                nc.gpsimd.tensor_tensor(out=t1[:], in0=inv, in1=inh, op=ADD)
                t2 = sp.tile([64, 196], f32, tag="t2")
                nc.vector.tensor_tensor(out=t2[:], in0=avs[h][:, 1:197], in1=t1[:],
                                        op=ADD)
                dst = outT[m][kp:kp + 64, t0 + 1:t0 + 197]
                nc.vector.tensor_tensor(out=dst, in0=t2[:], in1=rb[:, 1:197],
                                        op=MUL)
                nc.vector.tensor_tensor(out=outT[m][kp:kp + 64, t0:t0 + 1],
                                        in0=avs[h][:, 0:1], in1=rb[:, 0:1],
                                        op=MUL)

        # ---- final projection yT = wprojT.T @ outT + bproj ----
        out_eng = [nc.sync, nc.scalar]
        for m in range(5):
            for ci, (o0, on) in enumerate(CH4):
                acc = pj.tile([128, on], f32, tag="pj")
                for c in range(5):
                    nc.tensor.matmul(
                        acc[:], wp[c][:, 128 * m:128 * (m + 1)],
                        outT[c][:, o0:o0 + on],
                        start=(c == 0), stop=(c == 4))
                ysb = sp.tile([128, on], f32, tag="ysb")
                nc.vector.tensor_scalar_add(out=ysb[:], in0=acc[:],
                                            scalar1=bpr[m][:])
                out_eng[(m * 4 + ci) % 2].dma_start(
                    out=yT_d[128 * m:128 * (m + 1), o0:o0 + on], in_=ysb[:])

    nc.compile()
    return nc


_NC_CACHE = None


def kernel(x, w_qkv, w_proj, b_proj, rpk_v, rpk_h, rpv_v, rpv_h):
    global _NC_CACHE
    from concourse.bass_utils import run_bass_kernel_spmd

    if _NC_CACHE is None:
        _NC_CACHE = _build_nc()
    nc = _NC_CACHE

    EF2vh, E2all, Sel2 = _consts()
    wqkvT = np.ascontiguousarray(np.asarray(w_qkv).T).astype(ml_dtypes.bfloat16)
    wprojT = np.ascontiguousarray(np.asarray(w_proj).T).astype(ml_dtypes.bfloat16)
    bproj = np.asarray(b_proj, np.float32).reshape(C, 1)
    rpk_v = np.asarray(rpk_v, np.float32)
    rpk_h = np.asarray(rpk_h, np.float32)
    rpv_v = np.asarray(rpv_v, np.float32)
    rpv_h = np.asarray(rpv_h, np.float32)
    rpkT2s = np.concatenate([rpk_v.T, rpk_h.T], axis=1)
    rpkT2 = np.zeros((128, 128), ml_dtypes.bfloat16)
    rpkT2[0:64, 0:30] = rpk_v.T.astype(ml_dtypes.bfloat16)
    rpkT2[64:128, 64:94] = rpk_v.T.astype(ml_dtypes.bfloat16)
    rpkT2h = np.zeros((128, 128), ml_dtypes.bfloat16)
    rpkT2h[0:64, 32:62] = rpk_h.T.astype(ml_dtypes.bfloat16)
    rpkT2h[64:128, 96:126] = rpk_h.T.astype(ml_dtypes.bfloat16)
    rpvs = np.zeros((78, 28 * 64), np.float32)
    for r in range(14):
        rpvs[0:14, r * 64:(r + 1) * 64] = rpv_v[15 - r:29 - r, :]
        rpvs[64:78, 896 + r * 64:896 + (r + 1) * 64] = rpv_h[15 - r:29 - r, :]
    rpvcls = np.tile((rpv_v[0] + rpv_h[0]).reshape(1, 64), (1, 10))

    bf = ml_dtypes.bfloat16
    shared = {
        "wqkvT": wqkvT, "wprojT": wprojT, "bproj": bproj,
        "rpkT2": rpkT2, "rpkT2h": rpkT2h, "rpvs": rpvs.astype(bf), "rpvcls": rpvcls.astype(bf),
        "EF2vh": EF2vh.astype(bf), "E2all": E2all.astype(bf),
        "Sel2": Sel2.astype(bf),
    }
    x = np.asarray(x, np.float32)
    in_maps = []
    for i in range(NCORES):
        xs = x[i * NB:(i + 1) * NB].reshape(NT, C)
        xT = np.ascontiguousarray(xs.T).astype(bf)
        in_maps.append(dict(shared, xT=xT))

    trace = bool(os.environ.get("BASS_KERNEL_TRACE"))
    kw = {}
    if trace:
        kw = dict(trace=True, tmpdir=os.environ.get("BASS_KERNEL_TRACE_DIR") or None)
    res = run_bass_kernel_spmd(nc, in_maps, core_ids=list(range(NCORES)), **kw)
    kernel.last_result = res

    y = np.empty((B, N, C), np.float32)
    for i in range(NCORES):
        y[i * NB:(i + 1) * NB] = res.results[i]["yT"][:, :NT].T.reshape(NB, N, C)
    # cls-query rel-pos-v correction (constant across batch)
    rep = np.tile((rpv_v[0] + rpv_h[0]).astype(np.float32), H)
    y[:, 0, :] += np.asarray(w_proj, np.float32) @ rep
    return y
